# revision 10
# baseline (speedup 1.0000x reference)
"""Trainium2 Bass kernel for nn_Attention_41678362640976.

ViT-style attention block with a CLS-row prior injection:
  LayerNorm -> QKV (no bias) -> per-head S = q k^T * d^-0.5
  -> CLS row replaced by softmax(S[0,1:]) + canny_prior + noise_prior
  -> full softmax -> attn @ v -> out proj (+bias).

Sharding: pure data-parallel over batch, one batch element per NeuronCore
(B == 8 == n_cores). Each core runs an identical single-core program.

Per-core dataflow (N=1025 tokens, D=768, H=12 heads, HD=64):
  A. LayerNorm on x tiles [128,768] (bn_stats/bn_aggr), normalize
     (x-mu)*rstd in one tensor_scalar, PE-transpose to x_norm^T bf16
     [128, 6, 1025] applying ln_w/ln_b on the PSUM->SBUF copy.
  B. qT,kT = (w_qkv tile)^T @ x_norm^T  -> [128, 6, 1025] bf16 (q scaled by
     D^-0.5); v in natural layout via x_norm^T-stationary matmuls ->
     vsb [128, 9, 12, 65] bf16 with a ones column at offset 64 (the ones
     column makes the AV matmul also emit softmax denominators).
  C. CLS-row handling for all heads at once via block-diagonal lhsT
     ([12]-wide matmuls), exp with free-dim accum for the first softmax,
     prior add, exp(u) transposed to columns by PE for the column-0 fixup.
  D. Per head and per 512-query chunk: S^T tiles = kT^T qT (K=64), exp on
     ACT -> E bf16; AV: O'^T[65, i] accumulated over j-tiles with the ones
     row giving sums s_i at PSUM partition 64; column-0 overwritten by the
     corrected CLS accumulation; sums row staged to SBUF then DMAd to DRAM;
     O' copied (with 64-partition shift for odd heads) into Osb
     [128, 6, 1025] float32r.
  E. Denominators: DRAM bounce reshapes sums to [96,129] for a multi-lane
     reciprocal, bounce back to [12,1025]; R = hsel^T @ recip (PE) expands
     per-head recips to [128, i]; Osb *= R.
  F. out = Osb^T @ w_out (float32r) + b_out, DMA out.
"""

import numpy as np

import concourse.bass as bass
import concourse.mybir as mybir
import bass_rust as _bass_rust
from concourse.tile import TileContext
from concourse.bass_utils import run_bass_kernel_spmd

P = 128
N = 1025          # tokens (CLS + 32*32 patches)
D = 768
H = 12
HD = 64
KT = 6            # contraction tiles of 128 over D
NT = 8            # full 128-token tiles; token 1024 handled separately
SCALE = float(D) ** -0.5
EPS = 1e-5
F32 = mybir.dt.float32
F32R = mybir.dt.float32r
BF16 = mybir.dt.bfloat16
AF = mybir.ActivationFunctionType
ALU = mybir.AluOpType

# i-chunks for PSUM-bank-sized matmul outputs over the first 1024 queries
CH2 = [(0, 512), (512, 512)]
CH3 = CH2 + [(1024, 1)]


def build_core_program():
    nc = bass.Bass()

    x_d = nc.dram_tensor("x", [N, D], F32, kind="ExternalInput")
    canny_d = nc.dram_tensor("canny", [1, 32, 32], F32, kind="ExternalInput")
    noise_d = nc.dram_tensor("noise", [32, 32], F32, kind="ExternalInput")
    lnw_d = nc.dram_tensor("ln_w", [D], F32, kind="ExternalInput")
    lnb_d = nc.dram_tensor("ln_b", [D], F32, kind="ExternalInput")
    wqkv_d = nc.dram_tensor("w_qkv", [D, 3 * D], F32, kind="ExternalInput")
    wout_d = nc.dram_tensor("w_out", [D, D], F32, kind="ExternalInput")
    bout_d = nc.dram_tensor("b_out", [D], F32, kind="ExternalInput")
    out_d = nc.dram_tensor("out", [N, D], F32, kind="ExternalOutput")

    with TileContext(nc) as tc:
        with (
            tc.tile_pool(name="persist", bufs=1) as pp,
            tc.tile_pool(name="once", bufs=1) as op,
            tc.tile_pool(name="work", bufs=2) as wp,
            tc.tile_pool(name="wq", bufs=8) as wqp,
            tc.tile_pool(name="ebuf", bufs=2) as ep,
            tc.tile_pool(name="dram", bufs=1, space="DRAM") as dp,
            tc.tile_pool(name="ps_t", bufs=2, space="PSUM") as ps_t,
            tc.tile_pool(name="ps_b", bufs=2, space="PSUM") as ps_b,
            tc.tile_pool(name="ps_s", bufs=2, space="PSUM") as ps_s,
            tc.tile_pool(name="ps_av", bufs=2, space="PSUM") as ps_av,
        ):
            # ---------------- persistent tiles ----------------
            xnT = pp.tile([P, KT, N], BF16, name="xnT")
            qT = pp.tile([P, KT, N], BF16, name="qT")
            kT = pp.tile([P, KT, N], BF16, name="kT")
            vsb = pp.tile([P, NT + 1, H, HD + 1], BF16, name="vsb")
            Osb = pp.tile([P, KT, N], F32R, name="Osb")
            wout_sb = pp.tile([P, KT, D], F32R, name="wout_sb")
            wvall = pp.tile([P, KT, 2, 384], BF16, name="wvall")
            brep = pp.tile([P, D], F32, name="brep")
            lnw_col = pp.tile([P, KT], F32, name="lnw_col")
            lnb_col = pp.tile([P, KT], F32, name="lnb_col")
            id128 = pp.tile([P, P], BF16, name="id128")
            id12 = pp.tile([H, H], BF16, name="id12")
            cnrep = pp.tile([H, N - 1], F32, name="cnrep")
            q0b = pp.tile([P, KT, H], BF16, name="q0b")
            k1024b = pp.tile([P, KT, H], BF16, name="k1024b")
            q1024b = pp.tile([P, KT, H], BF16, name="q1024b")
            # four 12-row f32 tensors packed at 32-aligned partition offsets
            f32pk = pp.tile([P, N], F32, name="f32pk")
            clsrow = f32pk[0:H, :]
            srecip = f32pk[64 : 64 + H, :]
            e1row = f32pk[96 : 96 + H, 0 : N - 1]
            srecip_r = pp.tile([H, N], F32R, name="srecip_r")
            # two 12-row bf16 tensors packed the same way
            bf16pk = pp.tile([64, N], BF16, name="bf16pk")
            expu = bf16pk[0:H, :]
            elast = bf16pk[32 : 32 + H, :]
            sum1 = pp.tile([H, 1], F32, name="sum1")
            recip1 = pp.tile([H, 1], F32, name="recip1")
            expUc = pp.tile([P, NT + 1, H], BF16, name="expUc")
            e1024 = pp.tile([P, NT, H], BF16, name="e1024")
            recip96 = pp.tile([96, 129], F32, name="recip96")
            r96 = pp.tile([96, 129], F32, name="r96")
            hsel = pp.tile([H, KT, P], F32R, name="hsel")
            ones_row = pp.tile([1, HD], F32R, name="ones_row")
            eps_col = pp.tile([P, 1], F32, name="eps_col")

            # DRAM scratch (pool tiles so Tile tracks the RAW deps)
            scr_s = dp.tile([H, 1032], F32, name="scr_s")
            scr_r = dp.tile([H, 1032], F32, name="scr_r")
            scr_cn = dp.tile([1, N - 1], F32, name="scr_cn")
            scr_el = dp.tile([H, N], BF16, name="scr_el")

            # ---------------- constants ----------------
            from concourse.masks import make_identity
            make_identity(nc, id128[:])
            make_identity(nc, id12[:])
            nc.sync.dma_start(lnw_col[:], lnw_d[:].rearrange("(k p) -> p k", p=P))
            nc.sync.dma_start(lnb_col[:], lnb_d[:].rearrange("(k p) -> p k", p=P))
            nc.sync.dma_start(brep[:], bout_d[None, :].to_broadcast((P, D)))
            nc.vector.memset(ones_row[:].bitcast(F32), 1.0)
            nc.vector.memset(eps_col[:], EPS)
            nc.vector.memset(hsel[:].bitcast(F32), 0.0)
            for h in range(H):
                nc.sync.dma_start(
                    hsel[h : h + 1, h // 2, (h % 2) * HD : (h % 2) * HD + HD],
                    ones_row[:],
                )
            # ones column of vsb (col 64 of each head slot)
            nc.vector.memset(vsb[:, :, :, HD : HD + 1], 1.0)

            # ---------------- canny/noise priors ----------------
            crow = op.tile([1, N - 1], F32, name="crow")
            nrow = op.tile([1, N - 1], F32, name="nrow")
            csum = op.tile([1, 1], F32, name="csum")
            nsum = op.tile([1, 1], F32, name="nsum")
            crcp = op.tile([1, 1], F32, name="crcp")
            nrcp = op.tile([1, 1], F32, name="nrcp")
            nc.sync.dma_start(crow[:], canny_d[:].rearrange("a b c -> a (b c)"))
            nc.sync.dma_start(nrow[:], noise_d[:].rearrange("b c -> (b c)")[None, :])
            nc.vector.reduce_sum(csum[:], crow[:], axis=mybir.AxisListType.X)
            nc.vector.tensor_scalar_add(csum[:], csum[:], float(N - 1))
            nc.vector.reciprocal(crcp[:], csum[:])
            nc.vector.reduce_sum(nsum[:], nrow[:], axis=mybir.AxisListType.X)
            nc.vector.reciprocal(nrcp[:], nsum[:])
            nc.vector.tensor_scalar(
                crow[:], crow[:], 1.0, crcp[:, 0:1], ALU.add, ALU.mult
            )
            nc.vector.tensor_scalar_mul(nrow[:], nrow[:], nrcp[:, 0:1])
            nc.vector.tensor_add(crow[:], crow[:], nrow[:])
            nc.sync.dma_start(scr_cn[:], crow[:])
            nc.sync.dma_start(cnrep[:], scr_cn[:].to_broadcast((H, N - 1)))

            # ---------------- A: LayerNorm + transpose ----------------
            for tt in range(NT + 1):
                rows = P if tt < NT else 1
                xt = wp.tile([P, D], F32, name="xt")
                nc.sync.dma_start(xt[:rows], x_d[tt * P : tt * P + rows, :])
                stats = wp.tile([P, 2, 6], F32, name="stats")
                mv = wp.tile([P, 2], F32, name="mv")
                nc.vector.bn_stats(stats[:rows, 0, :], xt[:rows, 0 : D // 2])
                nc.vector.bn_stats(stats[:rows, 1, :], xt[:rows, D // 2 : D])
                nc.vector.bn_aggr(mv[:rows], stats[:rows])
                std = wp.tile([P, 1], F32, name="std")
                rstd = wp.tile([P, 1], F32, name="rstd")
                nc.scalar.activation(
                    std[:rows], mv[:rows, 1:2], AF.Sqrt, bias=eps_col[:rows, 0:1]
                )
                nc.vector.reciprocal(rstd[:rows], std[:rows])
                xc = wp.tile([P, D], BF16, name="xc")
                nc.vector.tensor_scalar(
                    xc[:rows],
                    xt[:rows],
                    mv[:rows, 0:1],
                    rstd[:rows, 0:1],
                    ALU.subtract,
                    ALU.mult,
                )
                for kt in range(KT):
                    pst = ps_t.tile([P, P], BF16, name="pst", tag="ps_small")
                    nc.tensor.transpose(
                        pst[:, :rows],
                        xc[:rows, kt * P : (kt + 1) * P],
                        id128[:rows, :rows],
                    )
                    nc.vector.tensor_scalar(
                        xnT[:, kt, tt * P : tt * P + rows],
                        pst[:, :rows],
                        lnw_col[:, kt : kt + 1],
                        lnb_col[:, kt : kt + 1],
                        ALU.mult,
                        ALU.add,
                    )

            # ---------------- B: q,k projections (transposed out) ----------
            for mt in range(12):
                wcol = mt * P  # q tiles then k tiles (w_qkv cols 0..1536)
                wts = []
                for kt in range(KT):
                    wt = wqp.tile([P, P], BF16, name="wt", tag="wqk")
                    nc.gpsimd.dma_start(
                        wt[:], wqkv_d[kt * P : (kt + 1) * P, wcol : wcol + P]
                    )
                    wts.append(wt)
                for cs, cl in CH3:
                    pb = ps_b.tile([P, 512], F32, name="pb", tag="ps_big")
                    for kt in range(KT):
                        nc.tensor.matmul(
                            pb[:, :cl],
                            wts[kt][:],
                            xnT[:, kt, cs : cs + cl],
                            start=(kt == 0),
                            stop=(kt == KT - 1),
                        )
                    if mt < 6:
                        nc.vector.tensor_scalar_mul(
                            qT[:, mt, cs : cs + cl], pb[:, :cl], SCALE
                        )
                    else:
                        nc.vector.tensor_copy(kT[:, mt - 6, cs : cs + cl], pb[:, :cl])

            # ---------------- B2: v in natural layout ----------------
            for kt in range(KT):
                nc.gpsimd.dma_start(
                    wvall[:, kt],
                    wqkv_d[kt * P : (kt + 1) * P, 2 * D : 3 * D].rearrange(
                        "p (c f) -> p c f", c=2
                    ),
                )
            for tt in range(NT + 1):
                rows = P if tt < NT else 1
                for c2 in range(2):
                    pb = ps_b.tile([P, 512], F32, name="pb", tag="ps_big")
                    for kt in range(KT):
                        nc.tensor.matmul(
                            pb[:rows, :384],
                            xnT[:, kt, tt * P : tt * P + rows],
                            wvall[:, kt, c2, :],
                            start=(kt == 0),
                            stop=(kt == KT - 1),
                        )
                    nc.vector.tensor_copy(
                        vsb[:rows, tt, 6 * c2 : 6 * c2 + 6, 0:HD],
                        pb[:rows, :384].rearrange("p (h f) -> p h f", h=6),
                    )

            # ---------------- C: CLS row + last-token helpers ----------------
            for blk, src, col in (
                (q0b, qT, 0),
                (k1024b, kT, 1024),
                (q1024b, qT, 1024),
            ):
                nc.vector.memset(blk[:], 0.0)
                for h in range(H):
                    qb = (h % 2) * HD
                    nc.vector.tensor_copy(
                        blk[qb : qb + HD, h // 2, h : h + 1],
                        src[qb : qb + HD, h // 2, col : col + 1],
                    )

            # cls logits row for every head: [12, 1025]
            for cs, cl in CH3:
                pc = ps_t.tile([H, 512], F32, name="pc", tag="ps_small")
                for kt in range(KT):
                    nc.tensor.matmul(
                        pc[:, :cl],
                        q0b[:, kt, :],
                        kT[:, kt, cs : cs + cl],
                        start=(kt == 0),
                        stop=(kt == KT - 1),
                    )
                nc.vector.tensor_copy(clsrow[:, cs : cs + cl], pc[:, :cl])

            # E_last = exp(S^T[1024, :]) for every head (row j=1024)
            for cs, cl in CH3:
                pc = ps_t.tile([H, 512], F32, name="pc", tag="ps_small")
                for kt in range(KT):
                    nc.tensor.matmul(
                        pc[:, :cl],
                        k1024b[:, kt, :],
                        qT[:, kt, cs : cs + cl],
                        start=(kt == 0),
                        stop=(kt == KT - 1),
                    )
                nc.scalar.activation(elast[:, cs : cs + cl], pc[:, :cl], AF.Exp)
            nc.sync.dma_start(scr_el[:], elast[:])

            # E_1024 column (i=1024, j<1024) for every head: [128, 8, 12]
            for jt in range(NT):
                pc = ps_t.tile([P, H], F32, name="pe", tag="ps_small")
                for kt in range(KT):
                    nc.tensor.matmul(
                        pc[:],
                        kT[:, kt, jt * P : (jt + 1) * P],
                        q1024b[:, kt, :],
                        start=(kt == 0),
                        stop=(kt == KT - 1),
                    )
                nc.scalar.activation(e1024[:, jt, :], pc[:], AF.Exp)

            # first softmax over cls row cols 1..1024, plus priors
            nc.scalar.activation(e1row[:], clsrow[:, 1:N], AF.Exp, accum_out=sum1[:])
            nc.vector.reciprocal(recip1[:], sum1[:])
            nc.vector.tensor_scalar_mul(clsrow[:, 1:N], e1row[:], recip1[:, 0:1])
            nc.vector.tensor_add(clsrow[:, 1:N], clsrow[:, 1:N], cnrep[:])
            # exp of the patched row (col 0 keeps the original logit)
            nc.scalar.activation(expu[:], clsrow[:], AF.Exp)
            # transpose expu rows into columns [128, 9, 12]
            for jt in range(NT + 1):
                rows = P if jt < NT else 1
                pst = ps_t.tile([P, H], BF16, name="pu", tag="ps_small")
                nc.tensor.transpose(
                    pst[:rows, :], expu[:, jt * P : jt * P + rows], id12[:]
                )
                nc.vector.tensor_copy(expUc[:rows, jt, :], pst[:rows, :])

            # ---------------- D: attention per head ----------------
            for h in range(H):
                qb = (h % 2) * HD
                dtile = h // 2
                el_h = ep.tile([1, N], BF16, name="el_h", tag="el_h")
                nc.sync.dma_start(el_h[:], scr_el[h : h + 1, :])
                for cs, cl in CH2:
                    Etc = ep.tile([P, NT, 512], BF16, name="Etc", tag="Etc")
                    for jt in range(NT):
                        pss = ps_s.tile([P, 512], F32, name="pss", tag="pss")
                        nc.tensor.matmul(
                            pss[:, :cl],
                            kT[qb : qb + HD, dtile, jt * P : (jt + 1) * P],
                            qT[qb : qb + HD, dtile, cs : cs + cl],
                            start=True,
                            stop=True,
                        )
                        nc.scalar.activation(Etc[:, jt, :cl], pss[:, :cl], AF.Exp)
                    # AV with ones row: psum rows 0:64 = O'^T, row 64 = sums
                    pav = ps_av.tile([P, 512], F32, name="pav", tag="pav")
                    for jt in range(NT):
                        nc.tensor.matmul(
                            pav[0 : HD + 1, :cl],
                            vsb[:, jt, h, :],
                            Etc[:, jt, :cl],
                            start=(jt == 0),
                            stop=False,
                        )
                    nc.tensor.matmul(
                        pav[0 : HD + 1, :cl],
                        vsb[0:1, NT, h, :],
                        el_h[0:1, cs : cs + cl],
                        start=False,
                        stop=True,
                    )
                    if cs == 0:
                        # overwrite column 0 with the corrected CLS column
                        for jt in range(NT):
                            nc.tensor.matmul(
                                pav[0 : HD + 1, 0:1],
                                vsb[:, jt, h, :],
                                expUc[:, jt, h : h + 1],
                                start=(jt == 0),
                                stop=False,
                            )
                        nc.tensor.matmul(
                            pav[0 : HD + 1, 0:1],
                            vsb[0:1, NT, h, :],
                            expUc[0:1, NT, h : h + 1],
                            start=False,
                            stop=True,
                        )
                    srow = wp.tile([HD + 1, 512], F32, name="srow", tag="srow")
                    nc.scalar.copy(srow[HD : HD + 1, :cl], pav[HD : HD + 1, :cl])
                    nc.sync.dma_start(
                        scr_s[h : h + 1, cs : cs + cl], srow[HD : HD + 1, :cl]
                    )
                    nc.vector.tensor_copy(
                        Osb[qb : qb + HD, dtile, cs : cs + cl], pav[0:HD, :cl]
                    )
                # query 1024 column
                pav1 = ps_t.tile([P, 1], F32, name="pav1", tag="ps_small")
                for jt in range(NT):
                    nc.tensor.matmul(
                        pav1[0 : HD + 1, :],
                        vsb[:, jt, h, :],
                        e1024[:, jt, h : h + 1],
                        start=(jt == 0),
                        stop=False,
                    )
                nc.tensor.matmul(
                    pav1[0 : HD + 1, :],
                    vsb[0:1, NT, h, :],
                    el_h[0:1, 1024:1025],
                    start=False,
                    stop=True,
                )
                srow1 = wp.tile([HD + 1, 1], F32, name="srow1", tag="srow1")
                nc.scalar.copy(srow1[HD : HD + 1, :], pav1[HD : HD + 1, :])
                nc.sync.dma_start(
                    scr_s[h : h + 1, 1024:1025], srow1[HD : HD + 1, :]
                )
                nc.vector.tensor_copy(
                    Osb[qb : qb + HD, dtile, 1024:1025], pav1[0:HD, :]
                )

            # ---------------- E: denominators ----------------
            nc.sync.dma_start(
                recip96[:], scr_s[:].rearrange("h (b c) -> (h b) c", c=129)
            )
            nc.vector.reciprocal(r96[:], recip96[:])
            nc.sync.dma_start(
                scr_r[:].rearrange("h (b c) -> (h b) c", c=129), r96[:]
            )
            nc.sync.dma_start(srecip[:], scr_r[:, 0:N])
            nc.vector.tensor_copy(srecip_r[:], srecip[:])
            for dtile in range(KT):
                for cs, cl in CH3:
                    pr = ps_b.tile([P, 512], F32, name="pr", tag="ps_big")
                    if cl % 2 == 0:
                        nc.tensor.matmul(
                            pr[:, :cl],
                            hsel[:, dtile, :],
                            srecip_r[:, cs : cs + cl],
                            start=True,
                            stop=True,
                        )
                    else:
                        nc.tensor.matmul(
                            pr[:, :cl],
                            hsel[:, dtile, :].bitcast(F32),
                            srecip_r[:, cs : cs + cl].bitcast(F32),
                            start=True,
                            stop=True,
                        )
                    nc.vector.tensor_tensor(
                        Osb[:, dtile, cs : cs + cl],
                        Osb[:, dtile, cs : cs + cl],
                        pr[:, :cl],
                        ALU.mult,
                    )

            # ---------------- F: out projection ----------------
            for kt in range(KT):
                wst = op.tile([P, D], F32, name="wst", tag=f"wst{kt}")
                nc.sync.dma_start(wst[:], wout_d[kt * P : (kt + 1) * P, :])
                nc.vector.tensor_copy(wout_sb[:, kt, :], wst[:])
            for tt in range(NT + 1):
                rows = P if tt < NT else 1
                for c2 in range(2):
                    po = ps_b.tile([P, 512], F32, name="po", tag="ps_big")
                    for kt in range(KT):
                        nc.tensor.matmul(
                            po[:rows, :384],
                            Osb[:, kt, tt * P : tt * P + rows],
                            wout_sb[:, kt, c2 * 384 : (c2 + 1) * 384],
                            start=(kt == 0),
                            stop=(kt == KT - 1),
                        )
                    ot = wp.tile([P, 384], F32, name="ot")
                    nc.vector.tensor_add(
                        ot[:rows],
                        po[:rows, :384],
                        brep[:rows, c2 * 384 : (c2 + 1) * 384],
                    )
                    nc.sync.dma_start(
                        out_d[tt * P : tt * P + rows, c2 * 384 : (c2 + 1) * 384],
                        ot[:rows],
                    )

    _bass_rust.generate_event_semaphores(nc)
    return nc


_NC_CACHE = None


def kernel(**inputs) -> np.ndarray:
    global _NC_CACHE
    x = np.ascontiguousarray(np.asarray(inputs["x"], dtype=np.float32))
    canny = np.ascontiguousarray(np.asarray(inputs["canny"], dtype=np.float32))
    noise = np.ascontiguousarray(np.asarray(inputs["noise"], dtype=np.float32))
    ln_w = np.ascontiguousarray(np.asarray(inputs["ln_w"], dtype=np.float32))
    ln_b = np.ascontiguousarray(np.asarray(inputs["ln_b"], dtype=np.float32))
    w_qkv = np.ascontiguousarray(np.asarray(inputs["w_qkv"], dtype=np.float32))
    w_out = np.ascontiguousarray(np.asarray(inputs["w_out"], dtype=np.float32))
    b_out = np.ascontiguousarray(np.asarray(inputs["b_out"], dtype=np.float32))

    B = x.shape[0]
    assert B == 8, f"expected batch 8, got {B}"

    if _NC_CACHE is None:
        _NC_CACHE = build_core_program()
    nc = _NC_CACHE

    in_maps = [
        {
            "x": x[b],
            "canny": canny[b],
            "noise": noise[b],
            "ln_w": ln_w,
            "ln_b": ln_b,
            "w_qkv": w_qkv,
            "w_out": w_out,
            "b_out": b_out,
        }
        for b in range(B)
    ]
    res = run_bass_kernel_spmd(nc, in_maps, core_ids=list(range(B)))
    out = np.stack([res.results[b]["out"] for b in range(B)], axis=0)
    return out.astype(np.float32)


# revision 13
# speedup vs baseline: 1.0083x; 1.0083x over previous
"""Trainium2 Bass kernel for nn_Attention_41678362640976.

ViT-style attention block with a CLS-row prior injection:
  LayerNorm -> QKV (no bias) -> per-head S = q k^T * d^-0.5
  -> CLS row replaced by softmax(S[0,1:]) + canny_prior + noise_prior
  -> full softmax -> attn @ v -> out proj (+bias).

Sharding: pure data-parallel over batch, one batch element per NeuronCore
(B == 8 == n_cores). Each core runs an identical single-core program.

Per-core dataflow (N=1025 tokens, D=768, H=12 heads, HD=64):
  A. LayerNorm on x tiles [128,768] (bn_stats/bn_aggr), normalize
     (x-mu)*rstd in one tensor_scalar, PE-transpose to x_norm^T bf16
     [128, 6, 1025] applying ln_w/ln_b on the PSUM->SBUF copy.
  B. qT,kT = (w_qkv tile)^T @ x_norm^T  -> [128, 6, 1025] bf16 (q scaled by
     D^-0.5); v in natural layout via x_norm^T-stationary matmuls ->
     vsb [128, 9, 12, 65] bf16 with a ones column at offset 64 (the ones
     column makes the AV matmul also emit softmax denominators).
  C. CLS-row handling for all heads at once via block-diagonal lhsT
     ([12]-wide matmuls), exp with free-dim accum for the first softmax,
     prior add, exp(u) transposed to columns by PE for the column-0 fixup.
  D. Per head and per 512-query chunk: S^T tiles = kT^T qT (K=64), exp on
     ACT -> E bf16; AV: O'^T[65, i] accumulated over j-tiles with the ones
     row giving sums s_i at PSUM partition 64; column-0 overwritten by the
     corrected CLS accumulation; sums row staged to SBUF then DMAd to DRAM;
     O' copied (with 64-partition shift for odd heads) into Osb
     [128, 6, 1025] float32r.
  E. Denominators: DRAM bounce reshapes sums to [96,129] for a multi-lane
     reciprocal, bounce back to [12,1025]; R = hsel^T @ recip (PE) expands
     per-head recips to [128, i]; Osb *= R.
  F. out = Osb^T @ w_out (float32r) + b_out, DMA out.
"""

import numpy as np

import concourse.bass as bass
import concourse.mybir as mybir
import bass_rust as _bass_rust
from concourse.tile import TileContext
from concourse.bass_utils import run_bass_kernel_spmd

P = 128
N = 1025          # tokens (CLS + 32*32 patches)
D = 768
H = 12
HD = 64
KT = 6            # contraction tiles of 128 over D
NT = 8            # full 128-token tiles; token 1024 handled separately
SCALE = float(D) ** -0.5
EPS = 1e-5
F32 = mybir.dt.float32
F32R = mybir.dt.float32r
BF16 = mybir.dt.bfloat16
AF = mybir.ActivationFunctionType
ALU = mybir.AluOpType

# i-chunks for PSUM-bank-sized matmul outputs over the first 1024 queries
CH2 = [(0, 512), (512, 512)]
CH3 = CH2 + [(1024, 1)]


def build_core_program():
    nc = bass.Bass()

    x_d = nc.dram_tensor("x", [N, D], F32, kind="ExternalInput")
    canny_d = nc.dram_tensor("canny", [1, 32, 32], F32, kind="ExternalInput")
    noise_d = nc.dram_tensor("noise", [32, 32], F32, kind="ExternalInput")
    lnw_d = nc.dram_tensor("ln_w", [D], F32, kind="ExternalInput")
    lnb_d = nc.dram_tensor("ln_b", [D], F32, kind="ExternalInput")
    wqkv_d = nc.dram_tensor("w_qkv", [D, 3 * D], F32, kind="ExternalInput")
    wout_d = nc.dram_tensor("w_out", [D, D], F32, kind="ExternalInput")
    bout_d = nc.dram_tensor("b_out", [D], F32, kind="ExternalInput")
    out_d = nc.dram_tensor("out", [N, D], F32, kind="ExternalOutput")

    with TileContext(nc) as tc:
        with (
            tc.tile_pool(name="persist", bufs=1) as pp,
            tc.tile_pool(name="once", bufs=1) as op,
            tc.tile_pool(name="work", bufs=2) as wp,
            tc.tile_pool(name="wq", bufs=8) as wqp,
            tc.tile_pool(name="ebuf", bufs=2) as ep,
            tc.tile_pool(name="dram", bufs=1, space="DRAM") as dp,
            tc.tile_pool(name="ps_t", bufs=2, space="PSUM") as ps_t,
            tc.tile_pool(name="ps_b", bufs=2, space="PSUM") as ps_b,
            tc.tile_pool(name="ps_s", bufs=2, space="PSUM") as ps_s,
            tc.tile_pool(name="ps_av", bufs=2, space="PSUM") as ps_av,
        ):
            # ---------------- persistent tiles ----------------
            xnT = pp.tile([P, KT, N], BF16, name="xnT")
            qT = pp.tile([P, KT, N], BF16, name="qT")
            kT = pp.tile([P, KT, N], BF16, name="kT")
            vsb = pp.tile([P, NT + 1, H, HD + 1], BF16, name="vsb")
            Osb = pp.tile([P, KT, N], F32R, name="Osb")
            wout_sb = pp.tile([P, KT, D], F32R, name="wout_sb")
            wvall = pp.tile([P, KT, 2, 384], BF16, name="wvall")
            brep = pp.tile([P, D], F32, name="brep")
            lnw_col = pp.tile([P, KT], F32, name="lnw_col")
            lnb_col = pp.tile([P, KT], F32, name="lnb_col")
            id128 = pp.tile([P, P], BF16, name="id128")
            id12 = pp.tile([H, H], BF16, name="id12")
            cnrep = pp.tile([H, N - 1], F32, name="cnrep")
            q0b = pp.tile([P, KT, H], BF16, name="q0b")
            k1024b = pp.tile([P, KT, H], BF16, name="k1024b")
            q1024b = pp.tile([P, KT, H], BF16, name="q1024b")
            # four 12-row f32 tensors packed at 32-aligned partition offsets
            f32pk = pp.tile([P, N], F32, name="f32pk")
            clsrow = f32pk[0:H, :]
            srecip = f32pk[64 : 64 + H, :]
            e1row = f32pk[96 : 96 + H, 0 : N - 1]
            srecip_r = pp.tile([H, N], F32R, name="srecip_r")
            # two 12-row bf16 tensors packed the same way
            bf16pk = pp.tile([64, N], BF16, name="bf16pk")
            expu = bf16pk[0:H, :]
            elast = bf16pk[32 : 32 + H, :]
            sum1 = pp.tile([H, 1], F32, name="sum1")
            recip1 = pp.tile([H, 1], F32, name="recip1")
            expUc = pp.tile([P, NT + 1, H], BF16, name="expUc")
            e1024 = pp.tile([P, NT, H], BF16, name="e1024")
            recip96 = pp.tile([96, 129], F32, name="recip96")
            r96 = pp.tile([96, 129], F32, name="r96")
            hsel = pp.tile([H, KT, P], F32R, name="hsel")
            ones_row = pp.tile([1, HD], F32R, name="ones_row")
            eps_col = pp.tile([P, 1], F32, name="eps_col")

            # DRAM scratch (pool tiles so Tile tracks the RAW deps)
            scr_s = dp.tile([H, 1032], F32, name="scr_s")
            scr_r = dp.tile([H, 1032], F32, name="scr_r")
            scr_cn = dp.tile([1, N - 1], F32, name="scr_cn")
            scr_el = dp.tile([H, N], BF16, name="scr_el")

            # ---------------- constants ----------------
            from concourse.masks import make_identity
            make_identity(nc, id128[:])
            make_identity(nc, id12[:])
            nc.sync.dma_start(lnw_col[:], lnw_d[:].rearrange("(k p) -> p k", p=P))
            nc.sync.dma_start(lnb_col[:], lnb_d[:].rearrange("(k p) -> p k", p=P))
            nc.vector.memset(ones_row[:].bitcast(F32), 1.0)
            nc.vector.memset(eps_col[:], EPS)
            # ones column of vsb (col 64 of each head slot)
            nc.vector.memset(vsb[:, :, :, HD : HD + 1], 1.0)

            # ---------------- A: LayerNorm + transpose ----------------
            for tt in range(NT + 1):
                rows = P if tt < NT else 1
                xt = wp.tile([P, D], F32, name="xt")
                nc.sync.dma_start(xt[:rows], x_d[tt * P : tt * P + rows, :])
                stats = wp.tile([P, 2, 6], F32, name="stats")
                mv = wp.tile([P, 2], F32, name="mv")
                nc.vector.bn_stats(stats[:rows, 0, :], xt[:rows, 0 : D // 2])
                nc.vector.bn_stats(stats[:rows, 1, :], xt[:rows, D // 2 : D])
                nc.vector.bn_aggr(mv[:rows], stats[:rows])
                std = wp.tile([P, 1], F32, name="std")
                rstd = wp.tile([P, 1], F32, name="rstd")
                nc.scalar.activation(
                    std[:rows], mv[:rows, 1:2], AF.Sqrt, bias=eps_col[:rows, 0:1]
                )
                nc.vector.reciprocal(rstd[:rows], std[:rows])
                xc = wp.tile([P, D], BF16, name="xc")
                nc.vector.tensor_scalar(
                    xc[:rows],
                    xt[:rows],
                    mv[:rows, 0:1],
                    rstd[:rows, 0:1],
                    ALU.subtract,
                    ALU.mult,
                )
                for kt in range(KT):
                    pst = ps_t.tile([P, P], BF16, name="pst", tag="ps_small")
                    nc.tensor.transpose(
                        pst[:, :rows],
                        xc[:rows, kt * P : (kt + 1) * P],
                        id128[:rows, :rows],
                    )
                    nc.scalar.activation(
                        xnT[:, kt, tt * P : tt * P + rows],
                        pst[:, :rows],
                        AF.Identity,
                        bias=lnb_col[:, kt : kt + 1],
                        scale=lnw_col[:, kt : kt + 1],
                    )

            # ---------------- B: q,k projections (transposed out) ----------
            for mt in range(12):
                wcol = mt * P  # q tiles then k tiles (w_qkv cols 0..1536)
                wtile = wqp.tile([P, KT, P], BF16, name="wtile", tag="wqk")
                nc.gpsimd.dma_start(
                    wtile[:],
                    wqkv_d[:, wcol : wcol + P].rearrange("(k p) c -> p k c", p=P),
                )
                for cs, cl in CH3:
                    pb = ps_b.tile([P, 512], F32, name="pb", tag="ps_big")
                    for kt in range(KT):
                        nc.tensor.matmul(
                            pb[:, :cl],
                            wtile[:, kt, :],
                            xnT[:, kt, cs : cs + cl],
                            start=(kt == 0),
                            stop=(kt == KT - 1),
                        )
                    if mt < 6:
                        nc.scalar.mul(qT[:, mt, cs : cs + cl], pb[:, :cl], SCALE)
                    else:
                        nc.scalar.copy(kT[:, mt - 6, cs : cs + cl], pb[:, :cl])

            # ---------------- B2: v in natural layout ----------------
            nc.gpsimd.dma_start(
                wvall[:],
                wqkv_d[:, 2 * D : 3 * D].rearrange(
                    "(k p) (c f) -> p k c f", p=P, c=2
                ),
            )
            for tt in range(NT + 1):
                rows = P if tt < NT else 1
                for c2 in range(2):
                    pb = ps_b.tile([P, 512], F32, name="pb", tag="ps_big")
                    for kt in range(KT):
                        nc.tensor.matmul(
                            pb[:rows, :384],
                            xnT[:, kt, tt * P : tt * P + rows],
                            wvall[:, kt, c2, :],
                            start=(kt == 0),
                            stop=(kt == KT - 1),
                        )
                    nc.vector.tensor_copy(
                        vsb[:rows, tt, 6 * c2 : 6 * c2 + 6, 0:HD],
                        pb[:rows, :384].rearrange("p (h f) -> p h f", h=6),
                    )

            # ---------------- canny/noise priors ----------------
            crow = op.tile([1, N - 1], F32, name="crow")
            nrow = op.tile([1, N - 1], F32, name="nrow")
            csum = op.tile([1, 1], F32, name="csum")
            nsum = op.tile([1, 1], F32, name="nsum")
            crcp = op.tile([1, 1], F32, name="crcp")
            nrcp = op.tile([1, 1], F32, name="nrcp")
            nc.sync.dma_start(crow[:], canny_d[:].rearrange("a b c -> a (b c)"))
            nc.sync.dma_start(nrow[:], noise_d[:].rearrange("b c -> (b c)")[None, :])
            nc.vector.reduce_sum(csum[:], crow[:], axis=mybir.AxisListType.X)
            nc.vector.tensor_scalar_add(csum[:], csum[:], float(N - 1))
            nc.vector.reciprocal(crcp[:], csum[:])
            nc.vector.reduce_sum(nsum[:], nrow[:], axis=mybir.AxisListType.X)
            nc.vector.reciprocal(nrcp[:], nsum[:])
            nc.vector.tensor_scalar(
                crow[:], crow[:], 1.0, crcp[:, 0:1], ALU.add, ALU.mult
            )
            nc.vector.tensor_scalar_mul(nrow[:], nrow[:], nrcp[:, 0:1])
            nc.vector.tensor_add(crow[:], crow[:], nrow[:])
            nc.sync.dma_start(scr_cn[:], crow[:])
            nc.sync.dma_start(cnrep[:], scr_cn[:].to_broadcast((H, N - 1)))

            for kt in range(KT):
                wst = op.tile([P, D], F32, name="wst", tag=f"wst{kt}")
                nc.sync.dma_start(wst[:], wout_d[kt * P : (kt + 1) * P, :])
                nc.vector.tensor_copy(wout_sb[:, kt, :], wst[:])
            nc.sync.dma_start(brep[:], bout_d[None, :].to_broadcast((P, D)))
            # ---------------- C: CLS row + last-token helpers ----------------
            for blk, src, col in (
                (q0b, qT, 0),
                (k1024b, kT, 1024),
                (q1024b, qT, 1024),
            ):
                nc.vector.memset(blk[:], 0.0)
                for h in range(H):
                    qb = (h % 2) * HD
                    nc.vector.tensor_copy(
                        blk[qb : qb + HD, h // 2, h : h + 1],
                        src[qb : qb + HD, h // 2, col : col + 1],
                    )

            # cls logits row for every head: [12, 1025]
            for cs, cl in CH3:
                pc = ps_t.tile([H, 512], F32, name="pc", tag="ps_small")
                for kt in range(KT):
                    nc.tensor.matmul(
                        pc[:, :cl],
                        q0b[:, kt, :],
                        kT[:, kt, cs : cs + cl],
                        start=(kt == 0),
                        stop=(kt == KT - 1),
                    )
                nc.vector.tensor_copy(clsrow[:, cs : cs + cl], pc[:, :cl])

            # E_last = exp(S^T[1024, :]) for every head (row j=1024)
            for cs, cl in CH3:
                pc = ps_t.tile([H, 512], F32, name="pc", tag="ps_small")
                for kt in range(KT):
                    nc.tensor.matmul(
                        pc[:, :cl],
                        k1024b[:, kt, :],
                        qT[:, kt, cs : cs + cl],
                        start=(kt == 0),
                        stop=(kt == KT - 1),
                    )
                nc.scalar.activation(elast[:, cs : cs + cl], pc[:, :cl], AF.Exp)
            nc.sync.dma_start(scr_el[:], elast[:])

            # E_1024 column (i=1024, j<1024) for every head: [128, 8, 12]
            for jt in range(NT):
                pc = ps_t.tile([P, H], F32, name="pe", tag="ps_small")
                for kt in range(KT):
                    nc.tensor.matmul(
                        pc[:],
                        kT[:, kt, jt * P : (jt + 1) * P],
                        q1024b[:, kt, :],
                        start=(kt == 0),
                        stop=(kt == KT - 1),
                    )
                nc.scalar.activation(e1024[:, jt, :], pc[:], AF.Exp)

            # first softmax over cls row cols 1..1024, plus priors
            nc.scalar.activation(e1row[:], clsrow[:, 1:N], AF.Exp, accum_out=sum1[:])
            nc.vector.reciprocal(recip1[:], sum1[:])
            nc.vector.tensor_scalar_mul(clsrow[:, 1:N], e1row[:], recip1[:, 0:1])
            nc.vector.tensor_add(clsrow[:, 1:N], clsrow[:, 1:N], cnrep[:])
            # exp of the patched row (col 0 keeps the original logit)
            nc.scalar.activation(expu[:], clsrow[:], AF.Exp)
            # transpose expu rows into columns [128, 9, 12]
            for jt in range(NT + 1):
                rows = P if jt < NT else 1
                pst = ps_t.tile([P, H], BF16, name="pu", tag="ps_small")
                nc.tensor.transpose(
                    pst[:rows, :], expu[:, jt * P : jt * P + rows], id12[:]
                )
                nc.vector.tensor_copy(expUc[:rows, jt, :], pst[:rows, :])

            # ---------------- D: attention per head ----------------
            for h in range(H):
                qb = (h % 2) * HD
                dtile = h // 2
                el_h = ep.tile([1, N], BF16, name="el_h", tag="el_h")
                nc.sync.dma_start(el_h[:], scr_el[h : h + 1, :])
                srow = wp.tile([HD + 1, N], F32, name="srow", tag="srow")
                for cs, cl in CH2:
                    Etc = ep.tile([P, NT, 512], BF16, name="Etc", tag="Etc")
                    for jt in range(NT):
                        pss = ps_s.tile([P, 512], F32, name="pss", tag="pss")
                        nc.tensor.matmul(
                            pss[:, :cl],
                            kT[qb : qb + HD, dtile, jt * P : (jt + 1) * P],
                            qT[qb : qb + HD, dtile, cs : cs + cl],
                            start=True,
                            stop=True,
                        )
                        nc.scalar.activation(Etc[:, jt, :cl], pss[:, :cl], AF.Exp)
                    # AV with ones row: psum rows 0:64 = O'^T, row 64 = sums
                    pav = ps_av.tile([P, 512], F32, name="pav", tag="pav")
                    for jt in range(NT):
                        nc.tensor.matmul(
                            pav[0 : HD + 1, :cl],
                            vsb[:, jt, h, :],
                            Etc[:, jt, :cl],
                            start=(jt == 0),
                            stop=False,
                        )
                    nc.tensor.matmul(
                        pav[0 : HD + 1, :cl],
                        vsb[0:1, NT, h, :],
                        el_h[0:1, cs : cs + cl],
                        start=False,
                        stop=True,
                    )
                    if cs == 0:
                        # overwrite column 0 with the corrected CLS column
                        for jt in range(NT):
                            nc.tensor.matmul(
                                pav[0 : HD + 1, 0:1],
                                vsb[:, jt, h, :],
                                expUc[:, jt, h : h + 1],
                                start=(jt == 0),
                                stop=False,
                            )
                        nc.tensor.matmul(
                            pav[0 : HD + 1, 0:1],
                            vsb[0:1, NT, h, :],
                            expUc[0:1, NT, h : h + 1],
                            start=False,
                            stop=True,
                        )
                    nc.vector.tensor_copy(
                        srow[HD : HD + 1, cs : cs + cl], pav[HD : HD + 1, :cl]
                    )
                    nc.vector.tensor_copy(
                        Osb[qb : qb + HD, dtile, cs : cs + cl], pav[0:HD, :cl]
                    )
                # query 1024 column
                pav1 = ps_t.tile([P, 1], F32, name="pav1", tag="ps_small")
                for jt in range(NT):
                    nc.tensor.matmul(
                        pav1[0 : HD + 1, :],
                        vsb[:, jt, h, :],
                        e1024[:, jt, h : h + 1],
                        start=(jt == 0),
                        stop=False,
                    )
                nc.tensor.matmul(
                    pav1[0 : HD + 1, :],
                    vsb[0:1, NT, h, :],
                    el_h[0:1, 1024:1025],
                    start=False,
                    stop=True,
                )
                nc.vector.tensor_copy(
                    srow[HD : HD + 1, 1024:1025], pav1[HD : HD + 1, :]
                )
                nc.sync.dma_start(scr_s[h : h + 1, 0:N], srow[HD : HD + 1, 0:N])
                nc.vector.tensor_copy(
                    Osb[qb : qb + HD, dtile, 1024:1025], pav1[0:HD, :]
                )

            nc.vector.memset(hsel[:].bitcast(F32), 0.0)
            for h in range(H):
                nc.sync.dma_start(
                    hsel[h : h + 1, h // 2, (h % 2) * HD : (h % 2) * HD + HD],
                    ones_row[:],
                )
            # ---------------- E: denominators ----------------
            nc.sync.dma_start(
                recip96[:], scr_s[:].rearrange("h (b c) -> (h b) c", c=129)
            )
            nc.vector.reciprocal(r96[:], recip96[:])
            nc.sync.dma_start(
                scr_r[:].rearrange("h (b c) -> (h b) c", c=129), r96[:]
            )
            nc.sync.dma_start(srecip[:], scr_r[:, 0:N])
            nc.vector.tensor_copy(srecip_r[:], srecip[:])
            for dtile in range(KT):
                for cs, cl in CH3:
                    pr = ps_b.tile([P, 512], F32, name="pr", tag="ps_big")
                    if cl % 2 == 0:
                        nc.tensor.matmul(
                            pr[:, :cl],
                            hsel[:, dtile, :],
                            srecip_r[:, cs : cs + cl],
                            start=True,
                            stop=True,
                        )
                    else:
                        nc.tensor.matmul(
                            pr[:, :cl],
                            hsel[:, dtile, :].bitcast(F32),
                            srecip_r[:, cs : cs + cl].bitcast(F32),
                            start=True,
                            stop=True,
                        )
                    nc.vector.tensor_tensor(
                        Osb[:, dtile, cs : cs + cl],
                        Osb[:, dtile, cs : cs + cl],
                        pr[:, :cl],
                        ALU.mult,
                    )

            # ---------------- F: out projection ----------------
            for tt in range(NT + 1):
                rows = P if tt < NT else 1
                ot = wp.tile([P, D], F32, name="ot")
                for c2 in range(2):
                    po = ps_b.tile([P, 512], F32, name="po", tag="ps_big")
                    for kt in range(KT):
                        nc.tensor.matmul(
                            po[:rows, :384],
                            Osb[:, kt, tt * P : tt * P + rows],
                            wout_sb[:, kt, c2 * 384 : (c2 + 1) * 384],
                            start=(kt == 0),
                            stop=(kt == KT - 1),
                        )
                    nc.vector.tensor_add(
                        ot[:rows, c2 * 384 : (c2 + 1) * 384],
                        po[:rows, :384],
                        brep[:rows, c2 * 384 : (c2 + 1) * 384],
                    )
                nc.sync.dma_start(out_d[tt * P : tt * P + rows, :], ot[:rows])

    _bass_rust.generate_event_semaphores(nc)
    return nc


_NC_CACHE = None


def kernel(**inputs) -> np.ndarray:
    global _NC_CACHE
    x = np.ascontiguousarray(np.asarray(inputs["x"], dtype=np.float32))
    canny = np.ascontiguousarray(np.asarray(inputs["canny"], dtype=np.float32))
    noise = np.ascontiguousarray(np.asarray(inputs["noise"], dtype=np.float32))
    ln_w = np.ascontiguousarray(np.asarray(inputs["ln_w"], dtype=np.float32))
    ln_b = np.ascontiguousarray(np.asarray(inputs["ln_b"], dtype=np.float32))
    w_qkv = np.ascontiguousarray(np.asarray(inputs["w_qkv"], dtype=np.float32))
    w_out = np.ascontiguousarray(np.asarray(inputs["w_out"], dtype=np.float32))
    b_out = np.ascontiguousarray(np.asarray(inputs["b_out"], dtype=np.float32))

    B = x.shape[0]
    assert B == 8, f"expected batch 8, got {B}"

    if _NC_CACHE is None:
        _NC_CACHE = build_core_program()
    nc = _NC_CACHE

    in_maps = [
        {
            "x": x[b],
            "canny": canny[b],
            "noise": noise[b],
            "ln_w": ln_w,
            "ln_b": ln_b,
            "w_qkv": w_qkv,
            "w_out": w_out,
            "b_out": b_out,
        }
        for b in range(B)
    ]
    res = run_bass_kernel_spmd(nc, in_maps, core_ids=list(range(B)))
    out = np.stack([res.results[b]["out"] for b in range(B)], axis=0)
    return out.astype(np.float32)
